# revision 15
# baseline (speedup 1.0000x reference)
"""Causal self-attention (B=2, T=2048, C=1024, H=16) on 8 trn2 NeuronCores.

Sharding: core c handles batch b = c//4 and head-group g = c%4 (4 heads,
256 qkv channels each).  c_attn is column-split, c_proj is row-split
(Megatron style); each core emits a partial [T, C] projection output and
the host sums the 4 partials per batch (+ b_proj).  No device collectives.

Per-core kernel (bf16 matmuls, fp32 accumulate):
  phase 1: Q^T,K^T  [256, T] = (x@Wq)^T via lhsT=W, rhs=x^T
           V'       [T, 4*65] = x@Wv (+ ones column per head for the
           softmax denominator)
  phase 2: per head pair, per 512-wide q chunk, per 128-wide k tile in
           DESCENDING ki order (diagonal tiles first):
           S^T [128k, 2*512q] = K_h^T.T @ Q_h^T for both heads into one
           2-bank PSUM tile; on diagonal tiles only columns [d:512) are
           computed/exp'd (d = within-block diagonal offset) and the
           causal mask multiply is a constant 128-wide strip.
           ONE exp on ScalarE (scale=1/8 folded in; safe without
           max-subtraction for N(0,1) scores), O^T accumulation
           [65, 512] per head (65th row = softmax denominator via V's
           ones column); descending order makes the first-issued OT
           matmul clear the bank (start=True) while later wider tiles
           overwrite-then-accumulate per element.
           normalize: 2 DVE reciprocals into one tile (rows 64/65), one
           K=2 selector matmul broadcasts both to 128 partitions, DVE
           copy to SBUF, DVE multiply + one bias add into y^T
  phase 3: partial = y^T.T @ Wproj_rows, DVE copy, DMA out (bf16)
Phases are emitted interleaved across q chunks so exp/DVE work overlaps
phase-1/3 matmuls; startup DMAs are per-chunk interleaved across both
HWDGE queues so the first matmul starts ~1us in.
"""

import os
import numpy as np
from contextlib import ExitStack

MMDT = os.environ.get("CSA_MMDT", "bf16")
ABL = set(os.environ.get("CSA_ABL", "").split(",")) - {""}  # timing ablations

B, T, C, NHEAD = 2, 2048, 1024, 16
HL = 4           # heads per core
HD = 64          # head dim
LQK = 512        # local q+k channels (2*HL*HD)
LV = 256         # local v channels
QC = 512         # q chunk width
NQ = T // QC     # 4 q chunks
NCC = C // 128   # 8 contraction chunks
NT = T // 128    # 16 row tiles
VW = 65          # V' width per head (64 + ones col)

_CACHE = {}


def _build_program(reps=1, mmdt="bf16"):
    import concourse.tile as tile
    from concourse import bacc, mybir

    F32 = mybir.dt.float32
    MDT = mybir.dt.float32r if mmdt == "f32r" else mybir.dt.bfloat16
    EXP = mybir.ActivationFunctionType.Exp

    nc = bacc.Bacc("TRN2", target_bir_lowering=False, debug=False, num_devices=8)

    xt = nc.dram_tensor("xt", [128, NCC, T], MDT, kind="ExternalInput").ap()
    wqk = nc.dram_tensor("wqk", [128, NCC, LQK], MDT, kind="ExternalInput").ap()
    wv = nc.dram_tensor("wv", [128, NCC, LV], MDT, kind="ExternalInput").ap()
    wp = nc.dram_tensor("wp", [128, 2, C], MDT, kind="ExternalInput").ap()
    bqk = nc.dram_tensor("bqk", [128, 4], F32, kind="ExternalInput").ap()
    bv = nc.dram_tensor("bv", [128, 2], F32, kind="ExternalInput").ap()
    # diag consts: [0:128) identity, [128:256) strict-lower-tri * -4000,
    # [256:320) all-ones
    diag = nc.dram_tensor("diag", [128, 320], MDT, kind="ExternalInput").ap()
    out = nc.dram_tensor("out", [T, C], MDT, kind="ExternalOutput").ap()

    with (
        tile.TileContext(nc) as tc,
        ExitStack() as ctx,
        nc.allow_low_precision(reason="bf16 storage/matmuls are deliberate"),
    ):
        # pools sized ~2 reps deep so rep R+1's phase-1 fillers can emit
        # during rep R's tail without head-of-line blocking the PE stream
        consts = ctx.enter_context(tc.tile_pool(name="consts", bufs=1))
        xpool = ctx.enter_context(tc.tile_pool(name="xp", bufs=4))
        qkpool = ctx.enter_context(tc.tile_pool(name="qk", bufs=16))
        vpool = ctx.enter_context(tc.tile_pool(name="v", bufs=32))
        ypool = ctx.enter_context(tc.tile_pool(name="y", bufs=8))
        ppool = ctx.enter_context(tc.tile_pool(name="pt", bufs=5))
        opool = ctx.enter_context(tc.tile_pool(name="obuf", bufs=4))
        rpool = ctx.enter_context(tc.tile_pool(name="rbc", bufs=3))
        ps1 = ctx.enter_context(tc.tile_pool(name="ps1", bufs=2, space="PSUM"))
        psS = ctx.enter_context(tc.tile_pool(name="psS", bufs=2, space="PSUM"))
        psO = ctx.enter_context(tc.tile_pool(name="psO", bufs=2, space="PSUM"))

        # consts that are NOT needed in the first ~5us go on the gpsimd
        # (SWDGE) queue; wqk/x chunks go interleaved on the two HWDGE
        # queues inside emit_ph1(0) so compute starts almost immediately.
        wqk_sb = consts.tile([128, NCC, LQK], MDT)
        bqk_sb = consts.tile([128, 4], F32)
        nc.gpsimd.dma_start(out=bqk_sb, in_=bqk)
        bv_sb = consts.tile([128, 2], F32)
        nc.gpsimd.dma_start(out=bv_sb, in_=bv)
        wv_sb = consts.tile([128, NCC, LV], MDT)
        nc.gpsimd.dma_start(out=wv_sb, in_=wv)
        wp_sb = consts.tile([128, 2, C], MDT)
        nc.gpsimd.dma_start(out=wp_sb, in_=wp)
        diag_sb = consts.tile([128, 320], MDT)
        nc.gpsimd.dma_start(out=diag_sb, in_=diag)
        ones_sb = diag_sb[:, 256:320]

        out_engs = [nc.sync, nc.scalar, nc.gpsimd]
        import collections

        # ---------------- filler management (shared across reps) ----------
        fillers = collections.deque()  # (label, generator)
        pend = [0.0]

        def fill(ns):
            pend[0] += ns
            while pend[0] > 0 and fillers:
                try:
                    pend[0] -= next(fillers[0][1])
                except StopIteration:
                    fillers.popleft()

        def ensure(label):
            while any(l == label for l, _ in fillers):
                try:
                    next(fillers[0][1])
                except StopIteration:
                    fillers.popleft()

        def drain(g):
            for _ in g:
                pass

        def make_ctx(R):
            # persistent activations, chunked for fine-grained deps
            qT = [qkpool.tile([128, 2, QC], MDT, tag="qT", name=f"qT{R}_{j}")
                  for j in range(NQ)]
            kT = [qkpool.tile([128, 2, QC], MDT, tag="kT", name=f"kT{R}_{j}")
                  for j in range(NQ)]
            vS = [vpool.tile([128, HL * VW], MDT, tag="vS", name=f"vS{R}_{t}")
                  for t in range(NT)]
            yT = [ypool.tile([128, 2, QC], MDT, tag="yT", name=f"yT{R}_{j}")
                  for j in range(NQ)]

            # ---------------- phase 1: qkv projections ----------------
            # generator: prologue (xp alloc + DMA) runs at creation; the
            # body yields after each whole accumulation chain (approx PE ns)
            def ph1_gen(j):
                xp = xpool.tile([128, NCC, QC], MDT, tag="xp", name=f"xp{R}_{j}")
                if j == 0 and R == 0:
                    # interleave wqk chunk c and x chunk c on opposite HWDGE
                    # queues: chunk c's matmul inputs land every ~1.4us
                    for c in range(NCC):
                        ea = nc.sync if c % 2 == 0 else nc.scalar
                        eb = nc.scalar if c % 2 == 0 else nc.sync
                        ea.dma_start(out=wqk_sb[:, c, :], in_=wqk[:, c, :])
                        eb.dma_start(out=xp[:, c, :], in_=xt[:, c, 0:QC])
                else:
                    x0 = QC * j
                    nc.sync.dma_start(out=xp[:, 0:4, :], in_=xt[:, 0:4, x0 : x0 + QC])
                    nc.sync.dma_start(out=xp[:, 4:8, :], in_=xt[:, 4:8, x0 : x0 + QC])

                def qk_chain(m):
                    ps = ps1.tile([128, QC], F32, tag="ps1")
                    for c in range(NCC):
                        nc.tensor.matmul(
                            ps,
                            lhsT=wqk_sb[:, c, 128 * m : 128 * (m + 1)],
                            rhs=xp[:, c, :],
                            start=(c == 0),
                            stop=(c == NCC - 1),
                        )
                    dst = (qT if m < 2 else kT)[j][:, m % 2, :]
                    nc.vector.tensor_scalar_add(dst, ps, bqk_sb[:, m : m + 1])

                def v_chain(t4):
                    tt = 4 * j + t4
                    ps = ps1.tile([128, QC], F32, tag="ps1")
                    psv = ps[:, 0:LV]
                    for c in range(NCC):
                        nc.tensor.matmul(
                            psv,
                            lhsT=xp[:, c, 128 * t4 : 128 * (t4 + 1)],
                            rhs=wv_sb[:, c, :],
                            start=(c == 0),
                            stop=(c == NCC - 1),
                        )
                    vst = vS[tt].rearrange("p (h e) -> p h e", e=VW)
                    nc.vector.tensor_copy(
                        vst[:, :, 0:HD],
                        psv.rearrange("p (h e) -> p h e", e=HD),
                    )
                    nc.vector.tensor_copy(
                        vst[:, :, HD : HD + 1],
                        ones_sb[:, 0:HL].rearrange("p (h e) -> p h e", e=1),
                    )

                def gen():
                    for m in (0, 2):
                        qk_chain(m)
                        yield 1700
                    for t4 in range(4):
                        v_chain(t4)
                        yield 850
                    for m in (1, 3):
                        qk_chain(m)
                        yield 1700

                return gen()

            # ---------------- phase 3: output projection ----------------
            def ph3_gen(j):
                def gen():
                    for tt in range(4 * j, 4 * j + 4):
                        for n in range(2):
                            ps = ps1.tile([128, QC], F32, tag="ps1")
                            for c2 in range(2):
                                nc.tensor.matmul(
                                    ps,
                                    lhsT=yT[j][:, c2,
                                               128 * (tt % 4) : 128 * (tt % 4 + 1)],
                                    rhs=wp_sb[:, c2, QC * n : QC * (n + 1)],
                                    start=(c2 == 0),
                                    stop=(c2 == 1),
                                )
                            ob = opool.tile([128, QC], MDT, tag="obuf")
                            nc.vector.tensor_copy(ob, ps)
                            eng = nc.scalar if (2 * tt + n) % 2 == 0 else nc.gpsimd
                            eng.dma_start(
                                out=out[128 * tt : 128 * (tt + 1),
                                        QC * n : QC * (n + 1)],
                                in_=ob,
                            )
                            yield 500
                return gen()

            # ---------------- phase 2: causal attention ----------------
            def emit_ph2_pair(j, pair):  # heads (2*pair, 2*pair+1)
                nk = 4 * j + 4
                oth = [
                    psO.tile([128, QC], F32, tag="psO",
                             name=f"ot{R}_{j}_{pair}_{hh}")
                    for hh in range(2)
                ]
                pts = [None] * nk

                def emit_ot(ki):
                    # diag tiles: cols < 128*dd are fully masked -> skip them
                    c0 = 128 * (ki - 4 * j) if ki >= 4 * j else 0
                    for hh in range(2):
                        h = 2 * pair + hh
                        nc.tensor.matmul(
                            oth[hh][0:VW, c0:QC],
                            lhsT=vS[ki][:, VW * h : VW * (h + 1)],
                            rhs=pts[ki][:, QC * hh + c0 : QC * (hh + 1)],
                            start=(ki == 0),
                            stop=(ki == nk - 1),
                        )

                for ki in range(nk):
                    # both heads' S^T into one 2-bank psum tile
                    sps = psS.tile([128, 2 * QC], F32, tag="psS",
                                   name=f"sps{R}_{j}_{pair}_{ki}")
                    isdiag = ki >= 4 * j
                    dd = ki - 4 * j
                    # diag tile dd: cols < 128*dd fully masked (skip); the
                    # boundary cuts through the 128-wide strip [c0, c0+128)
                    c0 = 128 * dd if isdiag else 0
                    for hh in range(2):
                        bp = 64 * hh
                        nc.tensor.matmul(
                            sps[:, QC * hh + c0 : QC * (hh + 1)],
                            lhsT=kT[ki // 4][bp : bp + 64, pair,
                                             128 * (ki % 4) : 128 * (ki % 4 + 1)],
                            rhs=qT[j][bp : bp + 64, pair, c0:QC],
                            start=True,
                            stop=not isdiag,
                        )
                    if isdiag:
                        # causal bias on the strip only: += -4000 where
                        # col' < p (strict lower tri), same pattern every dd
                        for hh in range(2):
                            nc.tensor.matmul(
                                sps[:, QC * hh + c0 : QC * hh + c0 + 128],
                                lhsT=diag_sb[:, 0:128],
                                rhs=diag_sb[:, 128:256],
                                start=False,
                                stop=True,
                            )
                    pt = ppool.tile([128, 2 * QC], MDT, tag="pt",
                                    name=f"pt{R}_{j}_{pair}_{ki}")
                    if c0:
                        nc.scalar.activation(
                            pt.rearrange("p (h c) -> p h c", c=QC)[:, :, c0:],
                            sps.rearrange("p (h c) -> p h c", c=QC)[:, :, c0:],
                            EXP, scale=0.125,
                        )
                    else:
                        nc.scalar.activation(pt, sps, EXP, scale=0.125)
                    pts[ki] = pt
                    # software pipeline: PE runs S(ki) two steps ahead of OT
                    if ki >= 2:
                        emit_ot(ki - 2)
                    fill(400)
                emit_ot(nk - 2)
                fill(400)
                emit_ot(nk - 1)

                # normalize + v-bias, write y^T
                rcs = rpool.tile([128, 2 * QC], MDT, tag="rc",
                                 name=f"rc{R}_{j}_{pair}")
                for hh in range(2):
                    nc.vector.reciprocal(rcs[64:65, QC * hh : QC * (hh + 1)],
                                         oth[hh][64:65, :])
                # PE chews fillers while the DVE reciprocals run
                fill(1500)
                # broadcast recip rows to partitions [64h:64h+64): K=1 matmuls
                rps = ps1.tile([128, QC], F32, tag="ps1",
                               name=f"rps{R}_{j}_{pair}")
                for hh in range(2):
                    nc.tensor.matmul(
                        rps[64 * hh : 64 * hh + 64, :],
                        lhsT=ones_sb[64:65, 0:64],
                        rhs=rcs[64:65, QC * hh : QC * (hh + 1)],
                        start=True, stop=True,
                    )
                rbc = rpool.tile([128, QC], MDT, tag="rbc",
                                 name=f"rbc{R}_{j}_{pair}")
                nc.vector.tensor_copy(rbc, rps)
                fill(600)
                for hh in range(2):
                    ydst = yT[j][64 * hh : 64 * hh + 64, pair, :]
                    nc.vector.tensor_mul(
                        ydst, oth[hh][0:64, :], rbc[64 * hh : 64 * hh + 64, :]
                    )
                nc.vector.tensor_scalar_add(
                    yT[j][:, pair, :], yT[j][:, pair, :],
                    bv_sb[:, pair : pair + 1],
                )

            return ph1_gen, ph3_gen, emit_ph2_pair

        # ---------------- interleaved emission, pipelined across reps -----
        # rep R+1's ph1(0) is appended as a filler during rep R's last pair
        # and rep R's ph3(3) feeds rep R+1's early ph2, so the PE stream
        # never drains between reps.
        ctxs = {}

        def get_ctx(R):
            if R not in ctxs:
                ctxs[R] = make_ctx(R)
            return ctxs[R]

        for R in range(reps):
            ph1_gen, ph3_gen, emit_ph2_pair = get_ctx(R)
            if R == 0:
                drain(ph1_gen(0))
            else:
                ensure(("ph1", 0, R))
            for jf in (1, 2, 3):
                fillers.append((("ph1", jf, R), ph1_gen(jf)))
            for j in range(NQ):
                for pair in range(2):
                    if pair == 0:
                        ensure(("ph1", j, R))
                    if j == 3 and pair == 1 and R + 1 < reps:
                        nxt_ph1, _, _ = get_ctx(R + 1)
                        fillers.append((("ph1", 0, R + 1), nxt_ph1(0)))
                    emit_ph2_pair(j, pair)
                    if pair == 1:
                        fillers.append((("ph3", j, R), ph3_gen(j)))
        while fillers:
            try:
                next(fillers[0][1])
            except StopIteration:
                fillers.popleft()

    nc.compile()
    return nc


def _host_inputs(x, w_attn, b_attn, w_proj, core, mmdt=None):
    """Per-core input arrays, pre-laid-out for the kernel."""
    mmdt = mmdt or MMDT
    if mmdt == "f32r":
        mdt = np.float32
    else:
        import ml_dtypes
        mdt = ml_dtypes.bfloat16
    b, g = core // 4, core % 4
    q0, k0, v0 = g * 256, C + g * 256, 2 * C + g * 256

    xtc = np.ascontiguousarray(
        x[b].T.reshape(NCC, 128, T).transpose(1, 0, 2)
    )  # [128, 8, 2048]
    wqk_cols = np.concatenate(
        [w_attn[:, q0 : q0 + 256], w_attn[:, k0 : k0 + 256]], axis=1
    )  # [1024, 512]
    wqkc = np.ascontiguousarray(wqk_cols.reshape(NCC, 128, LQK).transpose(1, 0, 2))
    wvc = np.ascontiguousarray(
        w_attn[:, v0 : v0 + 256].reshape(NCC, 128, LV).transpose(1, 0, 2)
    )
    wpc = np.ascontiguousarray(
        w_proj[g * 256 : (g + 1) * 256, :].reshape(2, 128, C).transpose(1, 0, 2)
    )
    bqkc = np.ascontiguousarray(
        np.stack(
            [
                b_attn[q0 : q0 + 128],
                b_attn[q0 + 128 : q0 + 256],
                b_attn[k0 : k0 + 128],
                b_attn[k0 + 128 : k0 + 256],
            ],
            axis=1,
        )
    )  # [128, 4]
    bvc = np.ascontiguousarray(
        np.stack([b_attn[v0 : v0 + 128], b_attn[v0 + 128 : v0 + 256]], axis=1)
    )  # [128, 2]
    ku = np.arange(128)[:, None]
    ident = np.eye(128, dtype=np.float32)
    cc = np.arange(128)[None, :]
    tri = np.where(cc >= ku, 0.0, -4000.0).astype(np.float32)  # -4000 if c < p
    ones = np.ones((128, 64), dtype=np.float32)
    diagc = np.ascontiguousarray(np.concatenate([ident, tri, ones], axis=1))
    return {
        "xt": xtc.astype(mdt),
        "wqk": wqkc.astype(mdt),
        "wv": wvc.astype(mdt),
        "wp": wpc.astype(mdt),
        "bqk": bqkc.astype(np.float32),
        "bv": bvc.astype(np.float32),
        "diag": diagc.astype(mdt),
    }


def _get_program(reps=1, mmdt=None):
    mmdt = mmdt or MMDT
    key = ("nc", reps, mmdt)
    if key not in _CACHE:
        _CACHE[key] = _build_program(reps, mmdt)
    return _CACHE[key]


def kernel(x, w_attn, b_attn, w_proj, b_proj):
    from concourse.bass_utils import run_bass_kernel_spmd

    x = np.asarray(x, np.float32)
    w_attn = np.asarray(w_attn, np.float32)
    b_attn = np.asarray(b_attn, np.float32)
    w_proj = np.asarray(w_proj, np.float32)
    b_proj = np.asarray(b_proj, np.float32)

    nc = _get_program()
    in_maps = [_host_inputs(x, w_attn, b_attn, w_proj, c) for c in range(8)]
    res = run_bass_kernel_spmd(nc, in_maps, core_ids=list(range(8)))
    partials = [res.results[c]["out"] for c in range(8)]
    out = np.empty((B, T, C), np.float32)
    for b in range(B):
        acc = np.sum(
            np.stack(partials[4 * b : 4 * b + 4]).astype(np.float64), axis=0
        )
        out[b] = (acc + b_proj.astype(np.float64)).astype(np.float32)
    return out

